# revision 32
# baseline (speedup 1.0000x reference)
"""Trainium2 Bass kernel for a 3-layer single-head attention model.

Model (per sequence, S=1024, D=512):
  x += positional_encoding
  3x: q,k,v = x@W{q,k,v}+b; attn = softmax(q k^T / sqrt(D)); x = relu((attn v)@Wo + bo)
  output = mean over sequence of final (S,1) column  -> (batch,)

Distribution: data-parallel over batch. 32 sequences -> 8 NeuronCores, 4 each.
Weights (~3MB) replicated to every core.

Key algebraic restructure: softmax normalization (a per-query scalar) commutes
with the output projection, so (softmax(s)@v)@Wo == (softmax(s)@(v@Wo)).
v@Wo = x@(Wv@Wo) + bv@Wo, and Wv@Wo / bv@Wo are input-independent weight
products folded on the host. This removes the entire output projection
matmul phase; for the last layer (Wo is [512,1]) it collapses attn@v from a
[1024,512] product to a [1024,1] one.

Per-core dataflow (all matmuls in float32r, 1 PE cycle/row at N=512):
  - activations kept transposed xT[d, s] in SBUF (4 tiles of [128, 1024])
  - qT,kT = W^T x (transposed layout, bias fused into ScalarE PSUM eviction)
  - vp = x@(Wv@Wo)+bv@Wo kept natural [s, d] (x is the stationary operand),
    bias row added on VectorE with a broadcast constant
  - scoresT[sk, sq] tiles; ScalarE evicts with exp(scale*x) fused (softmax
    max-subtraction skipped: |scores*scale| < ~4 for this model)
  - softmax row sums via ones-matrix matmul (partition-axis reduction on PE),
    giving sums broadcast across partitions; fast reciprocal on VectorE
  - attn@vp with unnormalized probs -> per-query normalization (VectorE) and
    relu+bias (ScalarE) applied on PSUM eviction, producing the next layer's
    xT directly
"""

import math
import os

import numpy as np

os.environ.setdefault("NEURON_RT_RESET_CORES", "1")

import concourse.bass as bass
import concourse.mybir as mybir
import concourse.tile as tile
from concourse import bacc, bass_utils

P = 128
D = 512
S = 1024
KD = D // P          # 4 chunks over model dim
KS = S // P          # 8 chunks over sequence dim
NH = 2               # process sq in halves of 512
SQH = S // NH
SEQ_PER_CORE = 4
N_CORES = 8
SCALE = 1.0 / math.sqrt(D)

F32 = mybir.dt.float32
F32R = mybir.dt.float32r
AF = mybir.ActivationFunctionType


def build_nc():
    nc = bacc.Bacc("TRN2", target_bir_lowering=False, debug=False)

    x_d = nc.dram_tensor("x", [SEQ_PER_CORE, D, S], F32, kind="ExternalInput").ap()
    pe_d = nc.dram_tensor("pe", [D, S], F32, kind="ExternalInput").ap()
    ones_d = nc.dram_tensor("ones", [P, P], F32R, kind="ExternalInput").ap()
    out_d = nc.dram_tensor("out", [SEQ_PER_CORE], F32, kind="ExternalOutput").ap()

    wd = []
    for li in range(3):
        dout = D if li < 2 else 1
        dv = D if li < 2 else 8   # layer-2 folded Wv@Wo padded to 8 cols (ISA min)
        wd.append({
            "wq": nc.dram_tensor(f"l{li}_wq", [D, D], F32R, kind="ExternalInput").ap(),
            "wk": nc.dram_tensor(f"l{li}_wk", [D, D], F32R, kind="ExternalInput").ap(),
            "wvo": nc.dram_tensor(f"l{li}_wvo", [D, dv], F32R,
                                  kind="ExternalInput").ap(),
            "bq": nc.dram_tensor(f"l{li}_bq", [D, 1], F32, kind="ExternalInput").ap(),
            "bk": nc.dram_tensor(f"l{li}_bk", [D, 1], F32, kind="ExternalInput").ap(),
            "bvo": nc.dram_tensor(f"l{li}_bvo", [P, dv], F32,
                                  kind="ExternalInput").ap(),
            "bo": nc.dram_tensor(f"l{li}_bo", [dout, 1], F32,
                                 kind="ExternalInput").ap(),
        })

    with tile.TileContext(nc) as tc:
        with (
            tc.tile_pool(name="const", bufs=1) as cpool,
            tc.tile_pool(name="w", bufs=1) as wpool,
            tc.tile_pool(name="xT", bufs=1) as xpool,
            tc.tile_pool(name="qk", bufs=1) as qkpool,
            tc.tile_pool(name="v", bufs=2) as vpool,
            tc.tile_pool(name="exp", bufs=2) as epool,
            tc.tile_pool(name="small", bufs=2) as spool,
            tc.tile_pool(name="tmp", bufs=4) as tpool,
            tc.tile_pool(name="load", bufs=1) as lpool,
            tc.tile_pool(name="psum", bufs=8, space="PSUM") as psum,
        ):
            # ---- startup constants (kept minimal: x+pe+wq are the critical
            # path to the first matmul; per-layer biases load lazily) ----
            pe_sb = []
            for k in range(KD):
                t = cpool.tile([P, S], F32, tag=f"pe{k}", name=f"pe{k}")
                nc.gpsimd.dma_start(t[:], pe_d[k * P:(k + 1) * P, :])
                pe_sb.append(t)
            ones_sb = cpool.tile([P, P], F32R, tag="ones")
            nc.gpsimd.dma_start(ones_sb[:], ones_d[:])
            out_sb = cpool.tile([1, SEQ_PER_CORE], F32, tag="out")
            bias_sb = [None, None, None]
            # preload the ScalarE exp table set during the initial DMA wait
            # (first ACTIVATE on a new set otherwise pays ~2.7us mid-pipeline)
            tpre = spool.tile([1, 8], F32, tag="tpre", bufs=1)
            nc.scalar.activation(tpre[:], ones_sb[0:1, 0:8].bitcast(F32),
                                 AF.Exp, scale=0.0)

            # ---- main loop ----
            for s_ in range(SEQ_PER_CORE):
                s = s_
                # load x and add positional encoding -> xT tiles [128, 1024]
                # (loaded and summed in column halves so the first QKV matmul
                #  group only waits on the first half of x)
                xt = []
                for k in range(KD):
                    t = xpool.tile([P, S], F32R, tag=f"xT{k}")
                    xt.append(t)

                def load_x_half(n, s=s_, xt=xt):
                    ns = slice(n * SQH, (n + 1) * SQH)
                    for k in range(KD):
                        xl = lpool.tile([P, SQH], F32, tag=f"xload{k % 2}",
                                        name="xl")
                        nc.sync.dma_start(xl[:], x_d[s, k * P:(k + 1) * P, ns])
                        # seq > 0: posenc add on idle GpSimd so it is not
                        # queued behind layer-2's VectorE backlog
                        eng = nc.vector if s == 0 else nc.gpsimd
                        eng.tensor_add(xt[k][:, ns], xl[:], pe_sb[k][:, ns])
                load_x_half(0)

                for li in range(3):
                    dout = D if li < 2 else 1
                    dv = D if li < 2 else 8
                    # stream this layer's weights (and, once, its biases)
                    w_sb = {}
                    for name in ("wq", "wk", "wvo"):
                        cols = D if name != "wvo" else dv
                        tiles = []
                        for k in range(KD):
                            t = wpool.tile([P, cols], F32R, tag=f"{name}{k}",
                                           name=name)
                            nc.sync.dma_start(t[:], wd[li][name][k * P:(k + 1) * P, :])
                            tiles.append(t)
                        w_sb[name] = tiles
                        if li == 0 and name == "wk":
                            load_x_half(1)
                    if bias_sb[li] is None:
                        ent = {"bq": [], "bk": [], "bo": []}
                        for k in range(KD):
                            bqc = cpool.tile([P, 1], F32, tag=f"l{li}bq{k}")
                            nc.gpsimd.dma_start(bqc[:], wd[li]["bq"][k * P:(k + 1) * P, :])
                            ent["bq"].append(bqc)
                            bkc = cpool.tile([P, 1], F32, tag=f"l{li}bk{k}")
                            nc.gpsimd.dma_start(bkc[:], wd[li]["bk"][k * P:(k + 1) * P, :])
                            ent["bk"].append(bkc)
                        for k in range(KD if li < 2 else 1):
                            boc = cpool.tile([P if li < 2 else 1, 1], F32,
                                             tag=f"l{li}bo{k}")
                            nc.gpsimd.dma_start(boc[:], wd[li]["bo"][k * P:(k + 1) * P, :]
                                                if li < 2 else wd[li]["bo"][0:1, :])
                            ent["bo"].append(boc)
                        bvo = cpool.tile([P, dv], F32, tag=f"l{li}bvo")
                        nc.gpsimd.dma_start(bvo[:], wd[li]["bvo"][:])
                        ent["bvo"] = bvo
                        bias_sb[li] = ent
                    bias = bias_sb[li]

                    # ---- qT, kT [d, s] ----
                    qt = [qkpool.tile([P, S], F32R, tag=f"qT{m}", name=f"qT{m}")
                          for m in range(KD)]
                    kt = [qkpool.tile([P, S], F32R, tag=f"kT{m}", name=f"kT{m}")
                          for m in range(KD)]
                    for n in range(NH):
                        ns = slice(n * SQH, (n + 1) * SQH)
                        for m in range(KD):
                            ms = slice(m * P, (m + 1) * P)
                            pq = psum.tile([P, SQH], F32, tag="ps")
                            for k in range(KD):
                                nc.tensor.matmul(pq[:], w_sb["wq"][k][:, ms],
                                                 xt[k][:, ns],
                                                 start=(k == 0), stop=(k == KD - 1))
                            nc.scalar.activation(qt[m][:, ns], pq[:], AF.Identity,
                                                 bias=bias["bq"][m][:])
                            pk = psum.tile([P, SQH], F32, tag="ps")
                            for k in range(KD):
                                nc.tensor.matmul(pk[:], w_sb["wk"][k][:, ms],
                                                 xt[k][:, ns],
                                                 start=(k == 0), stop=(k == KD - 1))
                            nc.scalar.activation(kt[m][:, ns], pk[:], AF.Identity,
                                                 bias=bias["bk"][m][:])

                    # ---- vp = x@(Wv Wo) + bv@Wo, natural [s, dout] ----
                    vt = []
                    for mb in range(KS):
                        mbs = slice(mb * P, (mb + 1) * P)
                        pv = psum.tile([P, dv], F32, tag="ps")
                        for k in range(KD):
                            nc.tensor.matmul(pv[:], xt[k][:, mbs], w_sb["wvo"][k][:],
                                             start=(k == 0), stop=(k == KD - 1))
                        t = vpool.tile([P, dv], F32R, tag=f"v{mb}", name=f"v{mb}")
                        nc.vector.tensor_add(t[:], pv[:], bias["bvo"][:])
                        vt.append(t)

                    if li == 2:
                        zrow = spool.tile([1, S], F32, tag="zrow", bufs=1)

                    for h in range(NH):
                        hs = slice(h * SQH, (h + 1) * SQH)
                        # ---- scoresT + exp (with VectorE chunk-sum chain) ----
                        et = []
                        acc = spool.tile([P, SQH], F32R, tag="acc")
                        for mb in range(KS):
                            mbs = slice(mb * P, (mb + 1) * P)
                            ps = psum.tile([P, SQH], F32, tag="ps")
                            for k in range(KD):
                                nc.tensor.matmul(ps[:], kt[k][:, mbs], qt[k][:, hs],
                                                 start=(k == 0), stop=(k == KD - 1))
                            e = epool.tile([P, SQH], F32R, tag=f"e{mb}", name=f"e{mb}")
                            nc.scalar.activation(e[:], ps[:], AF.Exp, scale=SCALE)
                            et.append(e)
                            if mb == 0:
                                nc.vector.tensor_copy(acc[:], e[:])
                            else:
                                nc.vector.tensor_add(acc[:], acc[:], e[:])
                        # ---- attn @ vp -> normalize -> relu -> next x ----
                        # (softmax denominator: one ones-matmul partition-
                        #  reduces the VectorE chunk sums, emitted under the
                        #  first attn@vp group so the PE never waits on it)
                        if li < 2:
                            if h == 0:
                                xnext = [xpool.tile([P, S], F32R, tag=f"xT{m}",
                                                    name=f"xn{m}") for m in range(KD)]
                            for m in range(KD):
                                ms = slice(m * P, (m + 1) * P)
                                pav = psum.tile([P, SQH], F32, tag="ps")
                                for mb in range(KS):
                                    nc.tensor.matmul(pav[:], vt[mb][:, ms], et[mb][:],
                                                     start=(mb == 0),
                                                     stop=(mb == KS - 1))
                                if m == 0:
                                    ss = psum.tile([P, SQH], F32, tag="ps")
                                    nc.tensor.matmul(ss[:], ones_sb[:], acc[:],
                                                     start=True, stop=True)
                                    rc = spool.tile([P, SQH], F32, tag="recip")
                                    nc.vector.reciprocal_approx_fast(out=rc[:],
                                                                     in_=ss[:])
                                tmp = tpool.tile([P, SQH], F32, tag="norm")
                                nc.vector.tensor_mul(tmp[:], pav[:], rc[:])
                                nc.scalar.activation(xnext[m][:, hs], tmp[:], AF.Relu,
                                                     bias=bias["bo"][m][:])
                        else:
                            po = psum.tile([1, SQH], F32, tag="ps")
                            for mb in range(KS):
                                nc.tensor.matmul(po[:], vt[mb][:, 0:1], et[mb][:],
                                                 start=(mb == 0), stop=(mb == KS - 1))
                            ss = psum.tile([P, SQH], F32, tag="ps")
                            nc.tensor.matmul(ss[:], ones_sb[:], acc[:],
                                             start=True, stop=True)
                            rc = spool.tile([P, SQH], F32, tag="recip")
                            nc.vector.reciprocal_approx_fast(out=rc[:], in_=ss[:])
                            nc.vector.tensor_mul(zrow[0:1, hs], po[:], rc[0:1, :])
                            # relu + running sum fused on ScalarE per half
                            if h == 0:
                                rrow = spool.tile([1, S], F32, tag="rrow", bufs=1)
                                yacc = [spool.tile([1, 1], F32, tag=f"yacc{i}",
                                                   name=f"yacc{i}")
                                        for i in range(NH)]
                            nc.scalar.activation(rrow[0:1, hs], zrow[0:1, hs],
                                                 AF.Relu, bias=bias_sb[2]["bo"][0][:],
                                                 accum_out=yacc[h][:])
                    if li < 2:
                        xt = xnext
                # final: mean over sequence
                ysum = spool.tile([1, 1], F32, tag="ysum")
                nc.vector.tensor_add(ysum[:], yacc[0][:], yacc[1][:])
                nc.scalar.mul(out_sb[0:1, s:s + 1], ysum[:], 1.0 / S)

            nc.sync.dma_start(out_d[:], out_sb[0:1, :])
    nc.compile()
    return nc


def _positional_encoding(seq_len, d_model):
    pos = np.arange(seq_len, dtype=np.float32)[:, None]
    div = np.exp(np.arange(0, d_model, 2, dtype=np.float32)
                 * (-math.log(10000.0) / d_model))
    ang = pos * div
    return np.stack([np.sin(ang), np.cos(ang)], axis=-1).reshape(
        seq_len, d_model).astype(np.float32)


def _make_in_maps(x, params):
    x = np.ascontiguousarray(np.asarray(x, dtype=np.float32))
    pe_t = np.ascontiguousarray(_positional_encoding(S, D).T)
    ones = np.ones((P, P), dtype=np.float32)
    common = {"pe": pe_t, "ones": ones}
    for li, p in enumerate(params):
        dout = D if li < 2 else 1
        wv = np.asarray(p["wv"], np.float64)
        wo = np.asarray(p["wo"], np.float64).reshape(D, dout)
        bv = np.asarray(p["bv"], np.float64).reshape(1, D)
        common[f"l{li}_wq"] = np.ascontiguousarray(np.asarray(p["wq"], np.float32))
        common[f"l{li}_wk"] = np.ascontiguousarray(np.asarray(p["wk"], np.float32))
        wvo = (wv @ wo).astype(np.float32)
        if dout == 1:
            wvo = np.concatenate([wvo, np.zeros((D, 7), np.float32)], axis=1)
        common[f"l{li}_wvo"] = np.ascontiguousarray(wvo)
        common[f"l{li}_bq"] = np.asarray(p["bq"], np.float32).reshape(D, 1).copy()
        common[f"l{li}_bk"] = np.asarray(p["bk"], np.float32).reshape(D, 1).copy()
        bvo = (bv @ wo).astype(np.float32)
        if dout == 1:
            bvo = np.concatenate([bvo, np.zeros((1, 7), np.float32)], axis=1)
        common[f"l{li}_bvo"] = np.ascontiguousarray(np.tile(bvo, (P, 1)))
        common[f"l{li}_bo"] = np.asarray(p["bo"], np.float32).reshape(dout, 1).copy()
    in_maps = []
    for c in range(N_CORES):
        xs = x[c * SEQ_PER_CORE:(c + 1) * SEQ_PER_CORE]          # [4, 1024, 512]
        xt = np.ascontiguousarray(xs.transpose(0, 2, 1))          # [4, 512, 1024]
        in_maps.append({"x": xt, **common})
    return in_maps


def _ensure_device_backend():
    """The bass kernel runs through PJRT on the axon-proxied NeuronCores.
    If the surrounding process pinned jax to cpu (common when running a
    reference model), re-point the default platform at axon."""
    import jax
    try:
        devs = jax.devices()
        if len(devs) >= N_CORES and devs[0].platform != "cpu":
            return
    except Exception:
        pass
    jax.config.update("jax_platforms", "axon")


def _run(x, params, trace=False, **kw):
    _ensure_device_backend()
    nc = build_nc()
    in_maps = _make_in_maps(x, params)
    last_err = None
    for _attempt in range(3):
        try:
            res = bass_utils.run_bass_kernel_spmd(
                nc, in_maps, core_ids=list(range(N_CORES)), trace=trace, **kw)
            break
        except Exception as e:  # transient device wedge -> retry
            last_err = e
    else:
        raise last_err
    out = np.concatenate([res.results[c]["out"] for c in range(N_CORES)])
    return out.astype(np.float32), res


def kernel(x, params):
    out, _ = _run(x, params)
    return out


# revision 33
# speedup vs baseline: 1.2024x; 1.2024x over previous
"""Trainium2 Bass kernel for a 3-layer single-head attention model.

Model (per sequence, S=1024, D=512):
  x += positional_encoding
  3x: q,k,v = x@W{q,k,v}+b; attn = softmax(q k^T / sqrt(D)); x = relu((attn v)@Wo + bo)
  output = mean over sequence of final (S,1) column  -> (batch,)

Distribution: data-parallel over batch. 32 sequences -> 8 NeuronCores, 4 each.
Weights (~3MB) replicated to every core.

Key algebraic restructure: softmax normalization (a per-query scalar) commutes
with the output projection, so (softmax(s)@v)@Wo == (softmax(s)@(v@Wo)).
v@Wo = x@(Wv@Wo) + bv@Wo, and Wv@Wo / bv@Wo are input-independent weight
products folded on the host. This removes the entire output projection
matmul phase; for the last layer (Wo is [512,1]) it collapses attn@v from a
[1024,512] product to a [1024,1] one.

Per-core dataflow (all matmuls in float32r, 1 PE cycle/row at N=512):
  - activations kept transposed xT[d, s] in SBUF (4 tiles of [128, 1024])
  - qT,kT = W^T x (transposed layout, bias fused into ScalarE PSUM eviction)
  - vp = x@(Wv@Wo)+bv@Wo kept natural [s, d] (x is the stationary operand),
    bias row added on VectorE with a broadcast constant
  - scoresT[sk, sq] tiles; ScalarE evicts with exp(scale*x) fused (softmax
    max-subtraction skipped: |scores*scale| < ~4 for this model)
  - softmax row sums via ones-matrix matmul (partition-axis reduction on PE),
    giving sums broadcast across partitions; fast reciprocal on VectorE
  - attn@vp with unnormalized probs -> per-query normalization (VectorE) and
    relu+bias (ScalarE) applied on PSUM eviction, producing the next layer's
    xT directly
"""

import math
import os

import numpy as np

os.environ.setdefault("NEURON_RT_RESET_CORES", "1")

import concourse.bass as bass
import concourse.mybir as mybir
import concourse.tile as tile
from concourse import bacc, bass_utils

P = 128
D = 512
S = 1024
KD = D // P          # 4 chunks over model dim
KS = S // P          # 8 chunks over sequence dim
NH = 2               # process sq in halves of 512
SQH = S // NH
SEQ_PER_CORE = 4
N_CORES = 8
SCALE = 1.0 / math.sqrt(D)

F32 = mybir.dt.float32
F32R = mybir.dt.float32r
AF = mybir.ActivationFunctionType


def build_nc():
    nc = bacc.Bacc("TRN2", target_bir_lowering=False, debug=False)

    x_d = nc.dram_tensor("x", [SEQ_PER_CORE, D, S], F32, kind="ExternalInput").ap()
    pe_d = nc.dram_tensor("pe", [D, S], F32, kind="ExternalInput").ap()
    ones_d = nc.dram_tensor("ones", [P, P], F32R, kind="ExternalInput").ap()
    out_d = nc.dram_tensor("out", [SEQ_PER_CORE], F32, kind="ExternalOutput").ap()

    wd = []
    for li in range(3):
        dout = D if li < 2 else 1
        dv = D if li < 2 else 8   # layer-2 folded Wv@Wo padded to 8 cols (ISA min)
        wd.append({
            "wq": nc.dram_tensor(f"l{li}_wq", [D, D], F32R, kind="ExternalInput").ap(),
            "wk": nc.dram_tensor(f"l{li}_wk", [D, D], F32R, kind="ExternalInput").ap(),
            "wvo": nc.dram_tensor(f"l{li}_wvo", [D, dv], F32R,
                                  kind="ExternalInput").ap(),
            "bq": nc.dram_tensor(f"l{li}_bq", [D, 1], F32, kind="ExternalInput").ap(),
            "bk": nc.dram_tensor(f"l{li}_bk", [D, 1], F32, kind="ExternalInput").ap(),
            "bvo": nc.dram_tensor(f"l{li}_bvo", [P, dv], F32,
                                  kind="ExternalInput").ap(),
            "bo": nc.dram_tensor(f"l{li}_bo", [dout, 1], F32,
                                 kind="ExternalInput").ap(),
        })

    with tile.TileContext(nc) as tc:
        with (
            tc.tile_pool(name="const", bufs=1) as cpool,
            tc.tile_pool(name="w", bufs=1) as wpool,
            tc.tile_pool(name="xT", bufs=1) as xpool,
            tc.tile_pool(name="qk", bufs=1) as qkpool,
            tc.tile_pool(name="v", bufs=2) as vpool,
            tc.tile_pool(name="exp", bufs=2) as epool,
            tc.tile_pool(name="small", bufs=2) as spool,
            tc.tile_pool(name="tmp", bufs=4) as tpool,
            tc.tile_pool(name="load", bufs=1) as lpool,
            tc.tile_pool(name="psum", bufs=8, space="PSUM") as psum,
        ):
            # ---- startup constants (kept minimal: x+pe+wq are the critical
            # path to the first matmul; per-layer biases load lazily) ----
            pe_sb = []
            for k in range(KD):
                t = cpool.tile([P, S], F32, tag=f"pe{k}", name=f"pe{k}")
                nc.gpsimd.dma_start(t[:], pe_d[k * P:(k + 1) * P, :])
                pe_sb.append(t)
            ones_sb = cpool.tile([P, P], F32R, tag="ones")
            nc.gpsimd.dma_start(ones_sb[:], ones_d[:])
            out_sb = cpool.tile([1, SEQ_PER_CORE], F32, tag="out")
            bias_sb = [None, None, None]
            # preload the ScalarE exp table set during the initial DMA wait
            # (first ACTIVATE on a new set otherwise pays ~2.7us mid-pipeline)
            tpre = spool.tile([1, 8], F32, tag="tpre", bufs=1)
            nc.scalar.activation(tpre[:], ones_sb[0:1, 0:8].bitcast(F32),
                                 AF.Exp, scale=0.0)

            # ---- main loop ----
            for s_ in range(SEQ_PER_CORE):
                s = s_
                # load x and add positional encoding -> xT tiles [128, 1024]
                # (loaded and summed in column halves so the first QKV matmul
                #  group only waits on the first half of x)
                xt = []
                for k in range(KD):
                    t = xpool.tile([P, S], F32R, tag=f"xT{k}")
                    xt.append(t)

                def load_x_half(n, s=s_, xt=xt):
                    ns = slice(n * SQH, (n + 1) * SQH)
                    for k in range(KD):
                        xl = lpool.tile([P, SQH], F32, tag=f"xload{k % 2}",
                                        name="xl")
                        nc.sync.dma_start(xl[:], x_d[s, k * P:(k + 1) * P, ns])
                        nc.vector.tensor_add(xt[k][:, ns], xl[:], pe_sb[k][:, ns])
                load_x_half(0)

                for li in range(3):
                    dout = D if li < 2 else 1
                    dv = D if li < 2 else 8
                    # stream this layer's weights (and, once, its biases)
                    w_sb = {}
                    for name in ("wq", "wk", "wvo"):
                        cols = D if name != "wvo" else dv
                        tiles = []
                        for k in range(KD):
                            t = wpool.tile([P, cols], F32R, tag=f"{name}{k}",
                                           name=name)
                            nc.sync.dma_start(t[:], wd[li][name][k * P:(k + 1) * P, :])
                            tiles.append(t)
                        w_sb[name] = tiles
                        if li == 0 and name == "wk":
                            load_x_half(1)
                    if bias_sb[li] is None:
                        ent = {"bq": [], "bk": [], "bo": []}
                        for k in range(KD):
                            bqc = cpool.tile([P, 1], F32, tag=f"l{li}bq{k}")
                            nc.gpsimd.dma_start(bqc[:], wd[li]["bq"][k * P:(k + 1) * P, :])
                            ent["bq"].append(bqc)
                            bkc = cpool.tile([P, 1], F32, tag=f"l{li}bk{k}")
                            nc.gpsimd.dma_start(bkc[:], wd[li]["bk"][k * P:(k + 1) * P, :])
                            ent["bk"].append(bkc)
                        for k in range(KD if li < 2 else 1):
                            boc = cpool.tile([P if li < 2 else 1, 1], F32,
                                             tag=f"l{li}bo{k}")
                            nc.gpsimd.dma_start(boc[:], wd[li]["bo"][k * P:(k + 1) * P, :]
                                                if li < 2 else wd[li]["bo"][0:1, :])
                            ent["bo"].append(boc)
                        bvo = cpool.tile([P, dv], F32, tag=f"l{li}bvo")
                        nc.gpsimd.dma_start(bvo[:], wd[li]["bvo"][:])
                        ent["bvo"] = bvo
                        bias_sb[li] = ent
                    bias = bias_sb[li]

                    # ---- qT, kT [d, s] ----
                    qt = [qkpool.tile([P, S], F32R, tag=f"qT{m}", name=f"qT{m}")
                          for m in range(KD)]
                    kt = [qkpool.tile([P, S], F32R, tag=f"kT{m}", name=f"kT{m}")
                          for m in range(KD)]
                    for n in range(NH):
                        ns = slice(n * SQH, (n + 1) * SQH)
                        for m in range(KD):
                            ms = slice(m * P, (m + 1) * P)
                            pq = psum.tile([P, SQH], F32, tag="ps")
                            for k in range(KD):
                                nc.tensor.matmul(pq[:], w_sb["wq"][k][:, ms],
                                                 xt[k][:, ns],
                                                 start=(k == 0), stop=(k == KD - 1))
                            nc.scalar.activation(qt[m][:, ns], pq[:], AF.Identity,
                                                 bias=bias["bq"][m][:])
                            pk = psum.tile([P, SQH], F32, tag="ps")
                            for k in range(KD):
                                nc.tensor.matmul(pk[:], w_sb["wk"][k][:, ms],
                                                 xt[k][:, ns],
                                                 start=(k == 0), stop=(k == KD - 1))
                            nc.scalar.activation(kt[m][:, ns], pk[:], AF.Identity,
                                                 bias=bias["bk"][m][:])

                    # ---- vp = x@(Wv Wo) + bv@Wo, natural [s, dout] ----
                    vt = []
                    for mb in range(KS):
                        mbs = slice(mb * P, (mb + 1) * P)
                        pv = psum.tile([P, dv], F32, tag="ps")
                        for k in range(KD):
                            nc.tensor.matmul(pv[:], xt[k][:, mbs], w_sb["wvo"][k][:],
                                             start=(k == 0), stop=(k == KD - 1))
                        t = vpool.tile([P, dv], F32R, tag=f"v{mb}", name=f"v{mb}")
                        nc.vector.tensor_add(t[:], pv[:], bias["bvo"][:])
                        vt.append(t)

                    if li == 2:
                        zrow = spool.tile([1, S], F32, tag="zrow", bufs=1)

                    for h in range(NH):
                        hs = slice(h * SQH, (h + 1) * SQH)
                        # ---- scoresT + exp (with VectorE chunk-sum chain) ----
                        et = []
                        acc = spool.tile([P, SQH], F32R, tag="acc")
                        for mb in range(KS):
                            mbs = slice(mb * P, (mb + 1) * P)
                            ps = psum.tile([P, SQH], F32, tag="ps")
                            for k in range(KD):
                                nc.tensor.matmul(ps[:], kt[k][:, mbs], qt[k][:, hs],
                                                 start=(k == 0), stop=(k == KD - 1))
                            e = epool.tile([P, SQH], F32R, tag=f"e{mb}", name=f"e{mb}")
                            nc.scalar.activation(e[:], ps[:], AF.Exp, scale=SCALE)
                            et.append(e)
                            if mb == 0:
                                nc.vector.tensor_copy(acc[:], e[:])
                            else:
                                nc.vector.tensor_add(acc[:], acc[:], e[:])
                        # ---- attn @ vp -> normalize -> relu -> next x ----
                        # (softmax denominator: one ones-matmul partition-
                        #  reduces the VectorE chunk sums, emitted under the
                        #  first attn@vp group so the PE never waits on it)
                        if li < 2:
                            if h == 0:
                                xnext = [xpool.tile([P, S], F32R, tag=f"xT{m}",
                                                    name=f"xn{m}") for m in range(KD)]
                            for m in range(KD):
                                ms = slice(m * P, (m + 1) * P)
                                pav = psum.tile([P, SQH], F32, tag="ps")
                                for mb in range(KS):
                                    nc.tensor.matmul(pav[:], vt[mb][:, ms], et[mb][:],
                                                     start=(mb == 0),
                                                     stop=(mb == KS - 1))
                                if m == 0:
                                    ss = psum.tile([P, SQH], F32, tag="ps")
                                    nc.tensor.matmul(ss[:], ones_sb[:], acc[:],
                                                     start=True, stop=True)
                                    rc = spool.tile([P, SQH], F32, tag="recip")
                                    nc.vector.reciprocal_approx_fast(out=rc[:],
                                                                     in_=ss[:])
                                tmp = tpool.tile([P, SQH], F32, tag="norm")
                                nc.vector.tensor_mul(tmp[:], pav[:], rc[:])
                                nc.scalar.activation(xnext[m][:, hs], tmp[:], AF.Relu,
                                                     bias=bias["bo"][m][:])
                        else:
                            po = psum.tile([1, SQH], F32, tag="ps")
                            for mb in range(KS):
                                nc.tensor.matmul(po[:], vt[mb][:, 0:1], et[mb][:],
                                                 start=(mb == 0), stop=(mb == KS - 1))
                            ss = psum.tile([P, SQH], F32, tag="ps")
                            nc.tensor.matmul(ss[:], ones_sb[:], acc[:],
                                             start=True, stop=True)
                            rc = spool.tile([P, SQH], F32, tag="recip")
                            nc.vector.reciprocal_approx_fast(out=rc[:], in_=ss[:])
                            nc.vector.tensor_mul(zrow[0:1, hs], po[:], rc[0:1, :])
                            # relu + running sum fused on ScalarE per half
                            if h == 0:
                                rrow = spool.tile([1, S], F32, tag="rrow", bufs=1)
                                yacc = [spool.tile([1, 1], F32, tag=f"yacc{i}",
                                                   name=f"yacc{i}")
                                        for i in range(NH)]
                            nc.scalar.activation(rrow[0:1, hs], zrow[0:1, hs],
                                                 AF.Relu, bias=bias_sb[2]["bo"][0][:],
                                                 accum_out=yacc[h][:])
                    if li < 2:
                        xt = xnext
                # final: mean over sequence
                ysum = spool.tile([1, 1], F32, tag="ysum")
                nc.vector.tensor_add(ysum[:], yacc[0][:], yacc[1][:])
                nc.scalar.mul(out_sb[0:1, s:s + 1], ysum[:], 1.0 / S)

            nc.sync.dma_start(out_d[:], out_sb[0:1, :])
    nc.compile()
    return nc


def _positional_encoding(seq_len, d_model):
    pos = np.arange(seq_len, dtype=np.float32)[:, None]
    div = np.exp(np.arange(0, d_model, 2, dtype=np.float32)
                 * (-math.log(10000.0) / d_model))
    ang = pos * div
    return np.stack([np.sin(ang), np.cos(ang)], axis=-1).reshape(
        seq_len, d_model).astype(np.float32)


def _make_in_maps(x, params):
    x = np.ascontiguousarray(np.asarray(x, dtype=np.float32))
    pe_t = np.ascontiguousarray(_positional_encoding(S, D).T)
    ones = np.ones((P, P), dtype=np.float32)
    common = {"pe": pe_t, "ones": ones}
    for li, p in enumerate(params):
        dout = D if li < 2 else 1
        wv = np.asarray(p["wv"], np.float64)
        wo = np.asarray(p["wo"], np.float64).reshape(D, dout)
        bv = np.asarray(p["bv"], np.float64).reshape(1, D)
        common[f"l{li}_wq"] = np.ascontiguousarray(np.asarray(p["wq"], np.float32))
        common[f"l{li}_wk"] = np.ascontiguousarray(np.asarray(p["wk"], np.float32))
        wvo = (wv @ wo).astype(np.float32)
        if dout == 1:
            wvo = np.concatenate([wvo, np.zeros((D, 7), np.float32)], axis=1)
        common[f"l{li}_wvo"] = np.ascontiguousarray(wvo)
        common[f"l{li}_bq"] = np.asarray(p["bq"], np.float32).reshape(D, 1).copy()
        common[f"l{li}_bk"] = np.asarray(p["bk"], np.float32).reshape(D, 1).copy()
        bvo = (bv @ wo).astype(np.float32)
        if dout == 1:
            bvo = np.concatenate([bvo, np.zeros((1, 7), np.float32)], axis=1)
        common[f"l{li}_bvo"] = np.ascontiguousarray(np.tile(bvo, (P, 1)))
        common[f"l{li}_bo"] = np.asarray(p["bo"], np.float32).reshape(dout, 1).copy()
    in_maps = []
    for c in range(N_CORES):
        xs = x[c * SEQ_PER_CORE:(c + 1) * SEQ_PER_CORE]          # [4, 1024, 512]
        xt = np.ascontiguousarray(xs.transpose(0, 2, 1))          # [4, 512, 1024]
        in_maps.append({"x": xt, **common})
    return in_maps


def _ensure_device_backend():
    """The bass kernel runs through PJRT on the axon-proxied NeuronCores.
    If the surrounding process pinned jax to cpu (common when running a
    reference model), re-point the default platform at axon."""
    import jax
    try:
        devs = jax.devices()
        if len(devs) >= N_CORES and devs[0].platform != "cpu":
            return
    except Exception:
        pass
    jax.config.update("jax_platforms", "axon")


def _run(x, params, trace=False, **kw):
    _ensure_device_backend()
    nc = build_nc()
    in_maps = _make_in_maps(x, params)
    last_err = None
    for _attempt in range(3):
        try:
            res = bass_utils.run_bass_kernel_spmd(
                nc, in_maps, core_ids=list(range(N_CORES)), trace=trace, **kw)
            break
        except Exception as e:  # transient device wedge -> retry
            last_err = e
    else:
        raise last_err
    out = np.concatenate([res.results[c]["out"] for c in range(N_CORES)])
    return out.astype(np.float32), res


def kernel(x, params):
    out, _ = _run(x, params)
    return out
